# revision 1
# baseline (speedup 1.0000x reference)
"""AffineNet kernel v4: ap_gather 4-tap bilinear, instruction-count optimized.

Same algorithm as v3 (see kernel_v3 docstring) with:
- index pipeline computed once for all 4 rounds in wrapped layout [P, 512]
  using integer host maps HJW4/WCI4, bit-identical to the weights pipeline's
  HJD4/Wb expressions (same op structure and constants)
- theta-only precomputes hoisted out of the repeat loop
- s=1 calls combine only lane 0 (lane 1 weight is identically zero)
- L1 (mean over channels) in 4 instructions via a 3D-strided load + X-reduce
"""
import numpy as np

B, C, H, W = 8, 8, 256, 256
HW = H * W
P = 128
NI = 2048
NR = 4
RST = 130
NE = 131 * RST + 2  # 17032
MAGIC = 12582912.0

_cache = {}


def _build_l1(repeat=1):
    import concourse.bacc as bacc
    import concourse.bass as bass
    import concourse.mybir as mybir
    import concourse.tile as tile
    f32 = mybir.dt.float32
    Alu = mybir.AluOpType
    nc = bacc.Bacc("TRN2", target_bir_lowering=False, debug=False)
    xs = nc.dram_tensor("xs", [C, HW], f32, kind="ExternalInput")
    xbar = nc.dram_tensor("xbar", [1, HW], mybir.dt.bfloat16, kind="ExternalOutput")
    xb_t = xbar.ap().rearrange("one (p s) -> (one p) s", p=P)
    with tile.TileContext(nc) as tc:
        with tc.tile_pool(name="pl", bufs=2) as pool:
            xs_t = xs.ap().rearrange("c (p s) -> c p s", p=P)
            for _ in range(repeat):
                t = pool.tile([P, C, HW // P], f32, tag="t", name="t")
                for cch in range(C):
                    nc.sync.dma_start(t[:, cch, :], xs_t[cch])
                nc.vector.tensor_tensor(t[:, 0:4, :], t[:, 0:4, :], t[:, 4:8, :],
                                        op=Alu.add)
                nc.vector.tensor_tensor(t[:, 0:2, :], t[:, 0:2, :], t[:, 2:4, :],
                                        op=Alu.add)
                nc.vector.tensor_tensor(t[:, 0:1, :], t[:, 0:1, :], t[:, 1:2, :],
                                        op=Alu.add)
                rb = pool.tile([P, HW // P], mybir.dt.bfloat16, tag="rb", name="rb")
                nc.vector.tensor_scalar_mul(rb[:], t[:, 0, :], 1.0 / C)
                nc.sync.dma_start(xb_t, rb[:])
    nc.compile()
    return nc


def _build_l2(repeat=1):
    import concourse.bacc as bacc
    import concourse.bass as bass
    import concourse.mybir as mybir
    import concourse.tile as tile
    f32 = mybir.dt.float32
    bf16 = mybir.dt.bfloat16
    i16 = mybir.dt.int16
    Alu = mybir.AluOpType

    nc = bacc.Bacc("TRN2", target_bir_lowering=False, debug=False)
    dimg = nc.dram_tensor("img", [P, NE * 2], bf16, kind="ExternalInput")
    tht = nc.dram_tensor("tht", [P, 8], f32, kind="ExternalInput")
    dwb = nc.dram_tensor("wb", [P, NI], mybir.dt.uint8, kind="ExternalInput")
    dhjd = nc.dram_tensor("hjd", [P, NR * NI], mybir.dt.uint8, kind="ExternalInput")
    dwci = nc.dram_tensor("wci", [P, NR * P], mybir.dt.uint8, kind="ExternalInput")
    dhjw = nc.dram_tensor("hjw", [P, NR * P], mybir.dt.uint8, kind="ExternalInput")
    daux = nc.dram_tensor("aux", [P, 8], f32, kind="ExternalInput")
    res = nc.dram_tensor("res", [B, HW], bf16, kind="ExternalOutput")

    with tile.TileContext(nc) as tc:
        with (
            tc.tile_pool(name="const", bufs=1) as cpool,
            tc.tile_pool(name="wk", bufs=1) as wp,
            tc.tile_pool(name="gt", bufs=1) as gp,
        ):
            img = cpool.tile([P, NE, 2], bf16, name="img")
            th = cpool.tile([P, 8], f32, name="th")
            Wb = cpool.tile([P, NI], mybir.dt.uint8, name="Wb")
            HJD = cpool.tile([P, NR * NI], mybir.dt.uint8, name="HJD")
            WCI = cpool.tile([P, NR * P], mybir.dt.uint8, name="WCI")
            HJW = cpool.tile([P, NR * P], mybir.dt.uint8, name="HJW")
            aux = cpool.tile([P, 8], f32, name="aux")
            nc.sync.dma_start(img[:], dimg.ap().rearrange("p (e l) -> p e l", l=2))
            nc.sync.dma_start(th[:], tht[:])
            nc.sync.dma_start(Wb[:], dwb[:])
            nc.sync.dma_start(HJD[:], dhjd[:])
            nc.sync.dma_start(WCI[:], dwci[:])
            nc.sync.dma_start(HJW[:], dhjw[:])
            nc.sync.dma_start(aux[:], daux[:])
            a_ = th[:, 0:1]
            b_ = th[:, 1:2]
            c0 = th[:, 2:3]
            d_ = th[:, 3:4]
            e_ = th[:, 4:5]
            f0 = th[:, 5:6]
            s2 = aux[:, 4:5]
            t0a = aux[:, 5:6]
            t0b = aux[:, 6:7]

            acc = cpool.tile([P, NR * NI], bf16, name="acc")

            for _ in range(repeat):
                # ===== wrapped-layout index pipeline, all rounds at once =====
                NW = NR * P  # 512
                TW = wp.tile([P, NW], f32, tag="TW", name="TW")
                UW = wp.tile([P, NW], f32, tag="UW", name="UW")
                FW = wp.tile([P, NW], bf16, tag="FW", name="FW")
                HW0 = wp.tile([P, NW], bf16, tag="HW0", name="HW0")
                RW0 = wp.tile([P, NW], bf16, tag="RW0", name="RW0")
                RW1 = wp.tile([P, NW], bf16, tag="RW1", name="RW1")
                QW = wp.tile([P, NW], bf16, tag="QW", name="QW")
                IDW = gp.tile([P, 4 * NW], i16, tag="IDW", name="IDW")
                nc.vector.tensor_scalar(TW[:], HJW[:], e_, f0,
                                        op0=Alu.mult, op1=Alu.add)
                nc.vector.scalar_tensor_tensor(TW[:], WCI[:], d_, TW[:],
                                               op0=Alu.mult, op1=Alu.add)
                nc.vector.tensor_scalar(TW[:], TW[:], -2.0, 257.5,
                                        op0=Alu.max, op1=Alu.min)
                nc.vector.tensor_scalar(TW[:], TW[:], 0.499, MAGIC,
                                        op0=Alu.subtract, op1=Alu.add)
                nc.vector.tensor_scalar(FW[:], TW[:], MAGIC, None,
                                        op0=Alu.subtract)
                nc.vector.tensor_scalar(HW0[:], FW[:], 2.0, -255.0,
                                        op0=Alu.mult, op1=Alu.add)
                nc.vector.tensor_scalar(HW0[:], HW0[:], 0.0, 1.0,
                                        op0=Alu.max, op1=Alu.min)
                nc.vector.scalar_tensor_tensor(RW0[:], HW0[:], -128.0, FW[:],
                                               op0=Alu.mult, op1=Alu.add)
                nc.vector.tensor_scalar(HW0[:], FW[:], 2.0, -253.0,
                                        op0=Alu.mult, op1=Alu.add)
                nc.vector.tensor_scalar(HW0[:], HW0[:], 0.0, 1.0,
                                        op0=Alu.max, op1=Alu.min)
                nc.vector.scalar_tensor_tensor(RW1[:], HW0[:], -128.0, FW[:],
                                               op0=Alu.mult, op1=Alu.add)
                nc.vector.tensor_scalar(RW1[:], RW1[:], 1.0, None, op0=Alu.add)
                nc.vector.tensor_scalar(UW[:], HJW[:], b_, c0,
                                        op0=Alu.mult, op1=Alu.add)
                nc.vector.scalar_tensor_tensor(UW[:], WCI[:], a_, UW[:],
                                               op0=Alu.mult, op1=Alu.add)
                nc.vector.tensor_scalar(UW[:], UW[:], -2.0, 257.5,
                                        op0=Alu.max, op1=Alu.min)
                nc.vector.tensor_scalar(UW[:], UW[:], 0.5, 0.499,
                                        op0=Alu.mult, op1=Alu.subtract)
                nc.vector.tensor_scalar(QW[:], UW[:], MAGIC, MAGIC,
                                        op0=Alu.add, op1=Alu.subtract)
                # idx layout: [call, R, c] so each (R, call) slice is contiguous
                for half in range(2):
                    RW = (RW0, RW1)[half]
                    for s in range(2):
                        ci_ = half * 2 + s
                        IDF = wp.tile([P, NW], f32, tag="IDF", name="IDF")
                        nc.vector.scalar_tensor_tensor(
                            IDF[:], RW[:], 130.0, QW[:], op0=Alu.mult, op1=Alu.add)
                        nc.vector.tensor_scalar(IDF[:], IDF[:], float(1 + s), 0.0,
                                                op0=Alu.add, op1=Alu.max)
                        nc.scalar.copy(IDW[:, ci_ * NW:(ci_ + 1) * NW], IDF[:])

                # ===== per-round weights + gathers =====
                for R in range(NR):
                    TA = wp.tile([P, NI], f32, tag="TA", name="TA")
                    VP = wp.tile([P, NI], f32, tag="VP", name="VP")
                    F = wp.tile([P, NI], bf16, tag="F", name="F")
                    FY = wp.tile([P, NI], bf16, tag="FY", name="FY")
                    HB0 = wp.tile([P, NI], bf16, tag="HB0", name="HB0")
                    HB1 = wp.tile([P, NI], bf16, tag="HB1", name="HB1")
                    WQ0 = wp.tile([P, NI], bf16, tag="WQ0", name="WQ0")
                    HJ = HJD[:, R * NI:(R + 1) * NI]
                    # y chain
                    nc.vector.tensor_scalar(TA[:], HJ, e_, f0,
                                            op0=Alu.mult, op1=Alu.add)
                    nc.vector.scalar_tensor_tensor(VP[:], Wb[:], d_, TA[:],
                                                   op0=Alu.mult, op1=Alu.add)
                    nc.vector.tensor_scalar(VP[:], VP[:], -2.0, 257.5,
                                            op0=Alu.max, op1=Alu.min)
                    nc.vector.tensor_scalar(TA[:], VP[:], 0.499, MAGIC,
                                            op0=Alu.subtract, op1=Alu.add)
                    nc.vector.tensor_scalar(F[:], TA[:], MAGIC, None,
                                            op0=Alu.subtract)
                    nc.vector.tensor_tensor(FY[:], VP[:], F[:], op=Alu.subtract)
                    nc.vector.tensor_scalar(HB0[:], F[:], s2, t0a,
                                            op0=Alu.mult, op1=Alu.add)
                    nc.vector.tensor_scalar(HB0[:], HB0[:], 0.0, 1.0,
                                            op0=Alu.max, op1=Alu.min)
                    nc.vector.tensor_scalar(HB1[:], F[:], s2, t0b,
                                            op0=Alu.mult, op1=Alu.add)
                    nc.vector.tensor_scalar(HB1[:], HB1[:], 0.0, 1.0,
                                            op0=Alu.max, op1=Alu.min)
                    nc.vector.tensor_scalar(WQ0[:], FY[:], -1.0, 1.0,
                                            op0=Alu.mult, op1=Alu.add)
                    nc.vector.tensor_tensor(WQ0[:], WQ0[:], HB0[:], op=Alu.mult)
                    nc.vector.tensor_tensor(FY[:], FY[:], HB1[:], op=Alu.mult)
                    # x chain (VP reused for xp)
                    TX = wp.tile([P, NI], bf16, tag="TX", name="TX")
                    WX0 = wp.tile([P, NI], bf16, tag="WX0", name="WX0")
                    WX1 = wp.tile([P, NI], bf16, tag="WX1", name="WX1")
                    WX2 = wp.tile([P, NI], bf16, tag="WX2", name="WX2")
                    Q = wp.tile([P, NI], bf16, tag="WX2", name="Qx")
                    nc.vector.tensor_scalar(TA[:], HJ, b_, c0,
                                            op0=Alu.mult, op1=Alu.add)
                    nc.vector.scalar_tensor_tensor(VP[:], Wb[:], a_, TA[:],
                                                   op0=Alu.mult, op1=Alu.add)
                    nc.vector.tensor_scalar(VP[:], VP[:], -2.0, 257.5,
                                            op0=Alu.max, op1=Alu.min)
                    nc.vector.tensor_scalar(TA[:], VP[:], 0.5, 0.499,
                                            op0=Alu.mult, op1=Alu.subtract)
                    nc.vector.tensor_scalar(Q[:], TA[:], MAGIC, MAGIC,
                                            op0=Alu.add, op1=Alu.subtract)
                    nc.vector.scalar_tensor_tensor(TX[:], Q[:], -2.0, VP[:],
                                                   op0=Alu.mult, op1=Alu.add)
                    nc.vector.tensor_scalar(WX0[:], TX[:], 1.0, None, op0=Alu.min)
                    nc.vector.tensor_scalar(WX0[:], WX0[:], -1.0, 1.0,
                                            op0=Alu.mult, op1=Alu.add)
                    nc.vector.tensor_scalar(WX1[:], TX[:], -1.0, 2.0,
                                            op0=Alu.mult, op1=Alu.add)
                    nc.vector.tensor_tensor(WX1[:], TX[:], WX1[:], op=Alu.min)
                    nc.vector.tensor_scalar(WX2[:], TX[:], 1.0, 0.0,
                                            op0=Alu.subtract, op1=Alu.max)
                    # gathers + combine
                    PS = wp.tile([P, NI], bf16, tag="PS", name="PS")
                    PT = wp.tile([P, NI], bf16, tag="PT", name="PT")
                    ACCR = PT  # in-place: PT dead once ACCR is formed
                    for half in range(2):
                        WQ = (WQ0, FY)[half]
                        for s in range(2):
                            ci_ = half * 2 + s
                            OUT = gp.tile([P, NI, 2], bf16, tag="OUT", name="OUT")
                            idxs = IDW[:, (ci_ * NR + R) * P:(ci_ * NR + R + 1) * P]
                            nc.gpsimd.ap_gather(
                                OUT[:], img[:], idxs,
                                channels=P, num_elems=NE, d=2, num_idxs=NI)
                            if s == 0:
                                WAL = wp.tile([P, NI, 2], bf16, tag="WAL",
                                              name="WAL")
                                nc.vector.tensor_tensor(WAL[:, :, 0], WQ[:],
                                                        WX0[:], op=Alu.mult)
                                nc.vector.tensor_tensor(WAL[:, :, 1], WQ[:],
                                                        WX1[:], op=Alu.mult)
                                nc.vector.tensor_tensor(OUT[:], OUT[:], WAL[:],
                                                        op=Alu.mult)
                                with nc.allow_low_precision("bf16 partials"):
                                    nc.vector.tensor_reduce(
                                        (PS, PT)[half][:], OUT[:],
                                        axis=mybir.AxisListType.X, op=Alu.add)
                            else:
                                WAL = wp.tile([P, NI, 2], bf16, tag="WAL",
                                              name="WAL")
                                nc.vector.tensor_tensor(WAL[:, :, 0], WQ[:],
                                                        WX2[:], op=Alu.mult)
                                nc.vector.tensor_tensor(WAL[:, :, 1], OUT[:, :, 0],
                                                        WAL[:, :, 0], op=Alu.mult)
                                with nc.allow_low_precision("bf16 partials"):
                                    if half == 0:
                                        nc.vector.tensor_tensor(
                                            PS[:], PS[:], WAL[:, :, 1], op=Alu.add)
                                    else:
                                        nc.vector.tensor_tensor(
                                            ACCR[:], PT[:], WAL[:, :, 1], op=Alu.add)
                    with nc.allow_low_precision("bf16 partials"):
                        nc.vector.tensor_tensor(ACCR[:], ACCR[:], PS[:],
                                                op=Alu.add)
                    # reorder list order j=c*16+k -> pixel order k*128+c
                    srcap = ACCR[:]
                    inap = bass.AP(srcap.tensor, srcap.offset,
                                   [srcap.ap[0], [1, 16], [16, 128]])
                    nc.vector.tensor_scalar(acc[:, R * NI:(R + 1) * NI], inap,
                                            0.0, None, op0=Alu.add)
                # ===== merge halves & write out =====
                HNI = NR * NI // 2
                for ch in range(2):
                    ST1 = wp.tile([64, HNI], bf16, tag="ST1", name="ST1")
                    ST2 = wp.tile([64, HNI], bf16, tag="ST2", name="ST2")
                    sl = slice(ch * HNI, (ch + 1) * HNI)
                    for g in range(8):
                        nc.sync.dma_start(ST1[g * 8:(g + 1) * 8, :],
                                          acc[g * 16 + 8:g * 16 + 16, sl])
                        nc.sync.dma_start(ST2[g * 8:(g + 1) * 8, :],
                                          acc[g * 16:g * 16 + 8, sl])
                    with nc.allow_low_precision("bf16 result"):
                        nc.vector.tensor_tensor(ST2[:], ST2[:], ST1[:], op=Alu.add)
                    for g in range(8):
                        dst = bass.AP(res.ap().tensor, g * NI + ch * 2 * 16384,
                                      [[HW, 8], [16384, 2], [1, NI]])
                        nc.sync.dma_start(dst, ST2[g * 8:(g + 1) * 8, :])
    nc.compile()
    return nc


def _consts():
    import ml_dtypes
    bf = ml_dtypes.bfloat16
    j = np.arange(NI, dtype=np.int64)
    wbv = ((j % 16) & 1) * 128 + (j >> 4)
    jdv = (j % 16) >> 1
    p = np.arange(P)
    g = p // 16
    jm = p % 16
    hbase = (np.arange(NR) * 64)[None, :, None] + (g * 8)[:, None, None]
    hjd = (hbase + jdv[None, None, :]).reshape(P, NR * NI).astype(np.uint8)
    wb = np.broadcast_to(wbv.astype(np.uint8), (P, NI))
    c = np.arange(P)
    wci4 = np.broadcast_to(
        ((128 * (jm & 1))[:, None] + c[None, :]).astype(np.uint8)[:, None, :],
        (P, NR, P)).reshape(P, NR * P)
    hjw4 = ((np.arange(NR) * 64)[None, :, None] + (g * 8 + (jm >> 1))[:, None, None]
            ).astype(np.uint8)
    hjw4 = np.broadcast_to(hjw4, (P, NR, P)).reshape(P, NR * P)
    aux = np.zeros((P, 8), np.float32)
    hfv = (jm // 8).astype(np.float32)
    aux[:, 0] = hfv
    aux[:, 1] = 2 * hfv - 1
    aux[:, 2] = 1 - hfv
    aux[:, 4] = 4 * hfv - 2                  # s2: hf? +2 : -2
    aux[:, 5] = np.where(hfv > 0, -255.0, 256.0)
    aux[:, 6] = np.where(hfv > 0, -253.0, 254.0)
    return (np.ascontiguousarray(wb), np.ascontiguousarray(hjd),
            np.ascontiguousarray(wci4), np.ascontiguousarray(hjw4),
            np.ascontiguousarray(aux))


def _build_img(xbb):
    import ml_dtypes
    bf = ml_dtypes.bfloat16
    img = np.zeros((P, NE, 2), bf)
    xb = np.asarray(xbb).reshape(8, 256, 256)
    for jm in range(16):
        b, hfv = jm % 8, jm // 8
        half = xb[b, 128 * hfv:128 * hfv + 128, :].reshape(128, 128, 2)
        one = np.zeros((NE, 2), bf)
        for rm in range(128):
            one[rm * RST + 1:rm * RST + 129] = half[rm]
        img[jm::16] = one[None, :, :]
    return np.ascontiguousarray(img.reshape(P, NE * 2))


def _theta_vec(th):
    a, bb = float(th[0, 0]), float(th[0, 1])
    c0 = 127.5 * (float(th[0, 2]) - a - bb + 1.0)
    d, e = float(th[1, 0]), float(th[1, 1])
    f0 = 127.5 * (float(th[1, 2]) - d - e + 1.0)
    vec = np.array([a, bb, c0, d, e, f0, 0.0, 0.0], np.float32)
    return np.ascontiguousarray(np.broadcast_to(vec, (P, 8)))


def _prep_in2(xbb, theta):
    wb, hjd, wci4, hjw4, aux = _consts()
    img = _build_img(xbb)
    in2 = []
    for o in range(B):
        in2.append({"img": img, "tht": _theta_vec(theta[o]), "wb": wb,
                    "hjd": hjd, "wci": wci4, "hjw": hjw4, "aux": aux})
    return in2


def kernel(x, theta):
    from concourse import bass_utils
    x = np.ascontiguousarray(x, dtype=np.float32)
    theta = np.ascontiguousarray(theta, dtype=np.float32)
    if "l1" not in _cache:
        _cache["l1"] = _build_l1()
        _cache["l2"] = _build_l2()
    l1, l2 = _cache["l1"], _cache["l2"]
    cores = list(range(8))
    in1 = [{"xs": np.ascontiguousarray(x[b].reshape(C, HW))} for b in range(B)]
    r1 = bass_utils.run_bass_kernel_spmd(l1, in1, core_ids=cores)
    xbb = np.ascontiguousarray(
        np.stack([r1.results[b]["xbar"].reshape(HW) for b in range(B)]))
    r2 = bass_utils.run_bass_kernel_spmd(l2, _prep_in2(xbb, theta), core_ids=cores)
    out = np.empty((B, B, H, W), np.float32)
    for o in range(B):
        out[:, o] = np.asarray(r2.results[o]["res"]).astype(np.float32).reshape(B, H, W)
    return out



# revision 2
# speedup vs baseline: 21.0041x; 21.0041x over previous
"""AffineNet v2: tent-matmul bilinear warp, theta-clipped per-core programs.

Math: out[b,o,r,xo] = sum_yi sum_xi tent(yi-iy)*tent(xi-ix)*xbar[b,yi,xi],
iy = d*xo + e*r + Cy, ix = a*xo + b*r + Cx, tent(t)=max(0,1-|t|).
This reproduces 4-tap bilinear with zeros padding exactly.

Host computes a schedule from theta: per o, the rows/xo-windows/xi-spans/
y-ranges that can be nonzero; slots = (o, row, 128-wide xo window); batches
= up to 8 consecutive rows sharing a window index. Batches are greedy-
balanced over 8 cores; each core gets its own static program, run
concurrently via per-device cached jax.jit dispatch.

Device per slot: y-pass matmul (stationary = y-tent [yi,xo]), PSUM->SBUF
evac on ACT, x-tent multiply + reduce on DVE.
"""
import numpy as np

B, C, H, W = 8, 8, 256, 256
HW = H * W
P = 128

_cache = {}


def _nullctx():
    import contextlib
    return contextlib.nullcontext()


# ---------------------------------------------------------------- L1: mean
def _build_l1(repeat=1):
    import concourse.bacc as bacc
    import concourse.mybir as mybir
    import concourse.tile as tile
    f32 = mybir.dt.float32
    Alu = mybir.AluOpType
    nc = bacc.Bacc("TRN2", target_bir_lowering=False, debug=False)
    xs = nc.dram_tensor("xs", [C, HW], f32, kind="ExternalInput")
    xbar = nc.dram_tensor("xbar", [1, HW], mybir.dt.bfloat16, kind="ExternalOutput")
    xb_t = xbar.ap().rearrange("one (p s) -> (one p) s", p=P)
    with tile.TileContext(nc) as tc:
        with tc.tile_pool(name="pl", bufs=2) as pool:
            xs_t = xs.ap().rearrange("c (p s) -> c p s", p=P)
            with tc.For_i(0, repeat, 1):
                t = pool.tile([P, C, HW // P], f32, tag="t", name="t")
                for cch in range(C):
                    nc.sync.dma_start(t[:, cch, :], xs_t[cch])
                nc.vector.tensor_tensor(t[:, 0:4, :], t[:, 0:4, :], t[:, 4:8, :],
                                        op=Alu.add)
                nc.vector.tensor_tensor(t[:, 0:2, :], t[:, 0:2, :], t[:, 2:4, :],
                                        op=Alu.add)
                nc.vector.tensor_tensor(t[:, 0:1, :], t[:, 0:1, :], t[:, 1:2, :],
                                        op=Alu.add)
                rb = pool.tile([P, HW // P], mybir.dt.bfloat16, tag="rb", name="rb")
                nc.vector.tensor_scalar_mul(rb[:], t[:, 0, :], 1.0 / C)
                nc.sync.dma_start(xb_t, rb[:])
    nc.compile()
    return nc


# ------------------------------------------------------------- scheduling
def _interval(lo, hi, coef, off):
    """xo interval where lo < coef*xo + off < hi, xo real."""
    if abs(coef) < 1e-12:
        return (0.0, 255.0) if lo < off < hi else None
    x1 = (lo - off) / coef
    x2 = (hi - off) / coef
    if x1 > x2:
        x1, x2 = x2, x1
    return (x1, x2)


def _isect(a, b):
    if a is None or b is None:
        return None
    lo, hi = max(a[0], b[0]), min(a[1], b[1])
    return (lo, hi) if lo <= hi else None


def make_sched(theta):
    """Returns (core_batches, slotmap) where core_batches[c] is a list of
    batch dicts and slotmap[c] is a list of (o, r, xw) or None per slot."""
    theta = np.asarray(theta, np.float64)
    MARGIN = 0.75
    slots_by_o = []
    for o in range(8):
        a, bb, cc = theta[o, 0]
        d, e, f = theta[o, 1]
        Cx = 127.5 * (cc + 1.0 - a - bb)
        Cy = 127.5 * (f + 1.0 - d - e)
        rows = []
        for r in range(256):
            vx = bb * r + Cx
            vy = e * r + Cy
            iv = _isect(_interval(-1.0 - MARGIN, 256.0 + MARGIN, a, vx),
                        _interval(-1.0 - MARGIN, 256.0 + MARGIN, d, vy))
            iv = _isect(iv, (0.0, 255.0))
            if iv is None:
                rows.append(None)
                continue
            xlo = int(np.floor(iv[0]))
            xhi = int(np.ceil(iv[1]))
            xlo = max(0, xlo)
            xhi = min(255, xhi)
            if xhi < xlo:
                rows.append(None)
                continue
            nwin = max(1, int(np.ceil((xhi - xlo + 1) / 128.0)))
            wins = []
            for k in range(nwin):
                xw = xlo + 128 * k
                if xw + 128 > 256:
                    xw = 128
                xw = min(xw, max(0, xhi - 127))
                xw = max(0, min(128, xw))
                wins.append(xw)
            wins = sorted(set(wins))
            rows.append((a, d, vx, vy, wins))
        slots_by_o.append((rows, (a, bb, d, e, Cx, Cy)))

    # build batches: per (o, window index k), runs of consecutive rows
    batches = []
    for o in range(8):
        rows, (a, bb, d, e, Cx, Cy) = slots_by_o[o]
        maxk = max((len(rr[4]) for rr in rows if rr), default=0)
        for k in range(maxk):
            run = []
            for r in range(257):
                has = r < 256 and rows[r] is not None and len(rows[r][4]) > k
                if has:
                    run.append(r)
                if (not has or len(run) == 8) and run:
                    batches.append(_mk_batch(o, run, k, rows,
                                             (a, bb, d, e, Cx, Cy)))
                    run = [] if not has else run[8:]
    # greedy balance by cost
    batches.sort(key=lambda b: -b["cost"])
    core_batches = [[] for _ in range(8)]
    loads = [0.0] * 8
    for bt in batches:
        i = int(np.argmin(loads))
        core_batches[i].append(bt)
        loads[i] += bt["cost"]
    # stable order within core
    for cb in core_batches:
        cb.sort(key=lambda b: (b["o"], b["r0"]))
    slotmaps = []
    for cb in core_batches:
        sm = []
        for bt in cb:
            for s in range(8):
                if s < len(bt["rows_"]):
                    sm.append((bt["o"], bt["rows_"][s], bt["xws"][s]))
                else:
                    sm.append(None)
        slotmaps.append(sm)
    return core_batches, slotmaps


def _mk_batch(o, run, k, rows, params):
    a, bb, d, e, Cx, Cy = params
    xws, xilos = [], []
    ylo_all, yhi_all = 1e9, -1e9
    xspan = 0
    for r in run:
        _, _, vx, vy, wins = rows[r]
        xw = wins[k]
        # ix range over window
        ixs = [a * xw + vx, a * (xw + 127) + vx]
        xi0 = int(np.floor(min(ixs))) - 1
        xi1 = int(np.floor(max(ixs))) + 2
        xi0 = max(0, min(255, xi0))
        xi1 = max(0, min(255, xi1))
        iys = [d * xw + vy, d * (xw + 127) + vy]
        y0 = int(np.floor(min(iys))) - 1
        y1 = int(np.floor(max(iys))) + 2
        ylo_all = min(ylo_all, max(0, y0))
        yhi_all = max(yhi_all, min(255, y1))
        xws.append(xw)
        xilos.append(xi0)
        xspan = max(xspan, xi1 - xi0 + 1)
    xspan = min(256, max(8, int(np.ceil(xspan / 8.0) * 8)))
    for i in range(len(xilos)):
        xilos[i] = min(xilos[i], 256 - xspan)
    halves = []
    for h in (0, 1):
        lo = max(ylo_all, 128 * h)
        hi = min(yhi_all, 128 * h + 127)
        if lo <= hi:
            # partition-sliced matmul operands fault on HW; use full halves
            halves.append((h, 0, 128))
    nrows = len(run)
    cost = nrows * len(halves) * xspan
    return dict(o=o, r0=run[0], rows_=run, xws=xws, xilos=xilos,
                xspan=xspan, halves=halves, nrows=nrows, cost=cost)


# ------------------------------------------------------------- L2 program
def _build_l2(batches, repeat=1):
    import concourse.bacc as bacc
    import concourse.bass as bass
    import concourse.mybir as mybir
    import concourse.tile as tile
    f32 = mybir.dt.float32
    bf16 = mybir.dt.bfloat16
    Alu = mybir.AluOpType
    Act = mybir.ActivationFunctionType
    MS = bass.MemorySpace

    nc = bacc.Bacc("TRN2", target_bir_lowering=False, debug=False)
    nbat = max(1, len(batches))
    dimg = nc.dram_tensor("img", [B, HW], bf16, kind="ExternalInput")
    dtht = nc.dram_tensor("tht", [P, 64], f32, kind="ExternalInput")
    dxo8 = nc.dram_tensor("xo8", [P, 256], f32, kind="ExternalInput")
    dr8 = nc.dram_tensor("r8", [P, 8], f32, kind="ExternalInput")
    diop = nc.dram_tensor("iop", [P, 1], f32, kind="ExternalInput")
    dxw = nc.dram_tensor("xw", [P, nbat * 8], f32, kind="ExternalInput")
    dxi = nc.dram_tensor("xi", [P, nbat * 8], f32, kind="ExternalInput")
    res = nc.dram_tensor("res", [nbat * 8, B * P], bf16, kind="ExternalOutput")

    with tile.TileContext(nc) as tc:
        with (
            tc.tile_pool(name="const", bufs=1) as cpool,
            tc.tile_pool(name="wk", bufs=2) as wp,
            tc.tile_pool(name="st", bufs=2) as sp,
            tc.tile_pool(name="ps", bufs=2, space=MS.PSUM) as pp,
        ):
            V0 = cpool.tile([P, B, W], bf16, name="V0")
            V1 = cpool.tile([P, B, W], bf16, name="V1")
            th = cpool.tile([P, 64], f32, name="th")
            xo8 = cpool.tile([P, 256], f32, name="xo8")
            r8 = cpool.tile([P, 8], f32, name="r8")
            iop = cpool.tile([P, 1], f32, name="iop")
            iop1 = cpool.tile([P, 1], f32, name="iop1")
            xwt = cpool.tile([P, nbat * 8], f32, name="xwt")
            xit = cpool.tile([P, nbat * 8], f32, name="xit")
            nc.sync.dma_start(th[:], dtht[:])
            nc.sync.dma_start(xo8[:], dxo8[:])
            nc.sync.dma_start(r8[:], dr8[:])
            nc.sync.dma_start(iop[:], diop[:])
            nc.sync.dma_start(xwt[:], dxw[:])
            nc.sync.dma_start(xit[:], dxi[:])
            nc.vector.tensor_scalar(iop1[:], iop[:], 128.0, None, op0=Alu.add)
            for b in range(B):
                src = dimg.ap()
                nc.sync.dma_start(
                    V0[:, b, :], bass.AP(src.tensor, b * HW, [[W, P], [1, W]]))
                nc.sync.dma_start(
                    V1[:, b, :],
                    bass.AP(src.tensor, b * HW + P * W, [[W, P], [1, W]]))

            with tc.For_i(0, repeat, 1) if batches else _nullctx():
                for bi, bt in enumerate(batches):
                    o = bt["o"]
                    r0 = bt["r0"]
                    xspan = bt["xspan"]
                    halves = bt["halves"]
                    nrows = bt["nrows"]
                    a_ = th[:, 8 * o + 0:8 * o + 1]
                    b_ = th[:, 8 * o + 1:8 * o + 2]
                    Cx_ = th[:, 8 * o + 2:8 * o + 3]
                    d_ = th[:, 8 * o + 3:8 * o + 4]
                    e_ = th[:, 8 * o + 4:8 * o + 5]
                    Cy_ = th[:, 8 * o + 5:8 * o + 6]
                    XW8 = xwt[:, bi * 8:bi * 8 + 8]
                    XI8 = xit[:, bi * 8:bi * 8 + 8]

                    # per-batch scalars [P, 8]
                    sc = wp.tile([P, 4, 8], f32, tag="sc", name="sc")
                    # sc1 = r0 + s
                    nc.vector.tensor_scalar(sc[:, 1, :], r8[:], float(r0),
                                            None, op0=Alu.add)
                    # scy[s] = e*(r0+s) + Cy + d*xw_s
                    nc.vector.tensor_scalar(sc[:, 0, :], sc[:, 1, :], e_,
                                            Cy_, op0=Alu.mult, op1=Alu.add)
                    nc.vector.tensor_scalar(sc[:, 3, :], XW8, d_, None,
                                            op0=Alu.mult)
                    nc.vector.tensor_tensor(sc[:, 0, :], sc[:, 0, :],
                                            sc[:, 3, :], op=Alu.add)
                    # scx[s] = b*(r0+s) + Cx + a*xw_s - xilo_s
                    nc.vector.tensor_scalar(sc[:, 2, :], sc[:, 1, :], b_,
                                            Cx_, op0=Alu.mult, op1=Alu.add)
                    nc.vector.tensor_scalar(sc[:, 3, :], XW8, a_, None,
                                            op0=Alu.mult)
                    nc.vector.tensor_tensor(sc[:, 2, :], sc[:, 2, :],
                                            sc[:, 3, :], op=Alu.add)
                    nc.vector.tensor_tensor(sc[:, 2, :], sc[:, 2, :],
                                            XI8, op=Alu.subtract)
                    axp = wp.tile([P, 1], f32, tag="axp", name="axp")
                    nc.vector.tensor_scalar(axp[:], iop[:], a_, None,
                                            op0=Alu.mult)

                    # x tents: TX [P(xo-in-win), 8s, xspan]
                    t1 = wp.tile([P, 8, xspan], f32, tag="t1", name="t1")
                    xov = bass.AP(xo8[:].tensor, xo8[:].offset,
                                  [xo8[:].ap[0], [0, 8], [1, xspan]])
                    scxv = bass.AP(sc[:].tensor, sc[:, 2, :].offset,
                                   [sc[:].ap[0], [1, 8], [0, xspan]])
                    nc.vector.scalar_tensor_tensor(t1[:], xov, axp[:], scxv,
                                                   op0=Alu.subtract,
                                                   op1=Alu.subtract)
                    txa = wp.tile([P, 8, xspan], bf16, tag="txa", name="txa")
                    TX = wp.tile([P, 8, xspan], bf16, tag="TX", name="TX")
                    nc.scalar.activation(txa[:], t1[:], Act.Abs)
                    nc.scalar.activation(TX[:], txa[:], Act.Relu,
                                         bias=1.0, scale=-1.0)

                    # y tents per half: WY [P(yi), 8s, 128]
                    t2 = wp.tile([P, 8, 128], f32, tag="t2", name="t2")
                    xov2 = bass.AP(xo8[:].tensor, xo8[:].offset,
                                   [xo8[:].ap[0], [0, 8], [1, 128]])
                    scyv = bass.AP(sc[:].tensor, sc[:, 0, :].offset,
                                   [sc[:].ap[0], [1, 8], [0, 128]])
                    nc.vector.scalar_tensor_tensor(t2[:], xov2, d_, scyv,
                                                   op0=Alu.mult, op1=Alu.add)
                    WYs = []
                    ty = wp.tile([P, 8, 128], f32, tag="ty", name="ty")
                    wya = wp.tile([P, 8, 128], bf16, tag="wya", name="wya")
                    for (h, ylo, ycnt) in halves:
                        WYh = wp.tile([P, 8, 128], bf16, tag=f"WY{h}",
                                      name=f"WY{h}")
                        nc.vector.tensor_scalar(ty[:], t2[:], -1.0,
                                                (iop, iop1)[h][:],
                                                op0=Alu.mult, op1=Alu.add)
                        nc.scalar.activation(wya[:], ty[:], Act.Abs)
                        nc.scalar.activation(WYh[:], wya[:], Act.Relu,
                                             bias=1.0, scale=-1.0)
                        WYs.append((h, ylo, ycnt, WYh))

                    nb = 8 if xspan <= 64 else (4 if xspan <= 128 else 2)
                    nchunk = B // nb
                    stage = sp.tile([P, 8, B], bf16, tag="stage", name="stage")
                    for s in range(nrows):
                        xilo = bt["xilos"][s]
                        # one PSUM bank (512 f32) per b-chunk, bank-aligned
                        ps = pp.tile([P, nchunk, 512], mybir.dt.float32,
                                     tag="ps", name="ps")
                        for hi, (h, ylo, ycnt, WYh) in enumerate(WYs):
                            lhsT = WYh[ylo:ylo + ycnt, s, :]
                            Vh = (V0, V1)[h]
                            for c in range(nchunk):
                                rhs = Vh[ylo:ylo + ycnt, c * nb:(c + 1) * nb,
                                         xilo:xilo + xspan]
                                nc.tensor.matmul(
                                    ps[:, c, 0:nb * xspan],
                                    lhsT, rhs,
                                    start=(hi == 0),
                                    stop=(hi == len(WYs) - 1))
                        N = wp.tile([P, B, xspan], bf16, tag="N", name="N")
                        psv = bass.AP(ps[:].tensor, ps[:].offset,
                                      [ps[:].ap[0], [512, nchunk],
                                       [1, nb * xspan]])
                        nc.scalar.copy(N[:], psv)
                        prod = wp.tile([P, B, xspan], bf16, tag="prod",
                                       name="prod")
                        txs = TX[:, s, :]
                        txb = bass.AP(txs.tensor, txs.offset,
                                      [txs.ap[0], [0, B], [1, xspan]])
                        nc.vector.tensor_tensor(prod[:], N[:], txb,
                                                op=Alu.mult)
                        with nc.allow_low_precision("bf16 out"):
                            nc.vector.tensor_reduce(
                                stage[:, s, :], prod[:],
                                axis=mybir.AxisListType.X, op=Alu.add)
                    dst = bass.AP(res.ap().tensor, bi * 8 * B * P,
                                  [[1, P], [B * P, nrows], [P, B]])
                    nc.sync.dma_start(dst, stage[:, 0:nrows, :])
    nc.compile()
    return nc


# ------------------------------------------------------------- host side
def _theta_consts(theta):
    v = np.zeros((8, 8), np.float32)
    for o in range(8):
        a, bb, c = (float(x) for x in theta[o, 0])
        d, e, f = (float(x) for x in theta[o, 1])
        v[o] = [a, bb, 127.5 * (c + 1.0 - a - bb),
                d, e, 127.5 * (f + 1.0 - d - e), 0.0, 0.0]
    return np.ascontiguousarray(np.broadcast_to(v.reshape(1, 64), (P, 64)))


def _host_consts():
    xo8 = np.broadcast_to(np.arange(256, dtype=np.float32)[None, :], (P, 256))
    r8 = np.broadcast_to(np.arange(8, dtype=np.float32)[None, :], (P, 8))
    iotap = np.arange(P, dtype=np.float32).reshape(P, 1)
    return (np.ascontiguousarray(xo8), np.ascontiguousarray(r8),
            np.ascontiguousarray(iotap))


def _prep_in2(xbb, theta, core_batches):
    xo8, r8, iotap = _host_consts()
    tht = _theta_consts(theta)
    img = np.ascontiguousarray(xbb)
    maps = []
    for cb in core_batches:
        nbat = max(1, len(cb))
        xw = np.zeros((nbat, 8), np.float32)
        xi = np.zeros((nbat, 8), np.float32)
        for i, bt in enumerate(cb):
            for s in range(bt["nrows"]):
                xw[i, s] = bt["xws"][s]
                xi[i, s] = bt["xilos"][s]
        maps.append({"img": img, "tht": tht, "xo8": xo8, "r8": r8,
                     "iop": iotap,
                     "xw": np.ascontiguousarray(
                         np.broadcast_to(xw.reshape(1, -1), (P, nbat * 8))),
                     "xi": np.ascontiguousarray(
                         np.broadcast_to(xi.reshape(1, -1), (P, nbat * 8)))})
    return maps


# --------------------------------------------------------- async runner
class Runner:
    """Cached per-device jit of a bass program; async dispatch."""

    def __init__(self, nc, device):
        import jax
        from concourse import bass2jax
        from concourse.bass2jax import _bass_exec_p, install_neuronx_cc_hook
        import concourse.mybir as mybir
        install_neuronx_cc_hook()
        in_names, out_names, out_avals, zero_outs = [], [], [], []
        for alloc in nc.m.functions[0].allocations:
            if not isinstance(alloc, mybir.MemoryLocationSet):
                continue
            name = alloc.memorylocations[0].name
            if alloc.kind == "ExternalInput":
                if name != "partition_id":
                    in_names.append(name)
            elif alloc.kind == "ExternalOutput":
                out_names.append(name)
                shape = tuple(alloc.tensor_shape)
                dtype = mybir.dt.np(alloc.dtype)
                out_avals.append(jax.core.ShapedArray(shape, dtype))
                zero_outs.append(np.zeros(shape, dtype))
        self.in_names = in_names
        self.out_names = out_names
        self.zero_outs = zero_outs
        self.device = device
        pid = "partition_id" if nc.partition_id_tensor is not None else None
        names2 = tuple(in_names + out_names + ([pid] if pid else []))

        def _body(*args):
            operands = list(args)
            if pid:
                operands.append(bass2jax.partition_id_tensor())
            outs = _bass_exec_p.bind(
                *operands, out_avals=tuple(out_avals),
                in_names=names2, out_names=tuple(out_names),
                lowering_input_output_aliases=(),
                sim_require_finite=True, sim_require_nnan=True, nc=nc)
            return tuple(outs)

        n_params = len(in_names)
        donate = tuple(range(n_params, n_params + len(out_names)))
        self.jitted = jax.jit(_body, donate_argnums=donate,
                              keep_unused=True, device=device)
        self._dev_inputs = None

    def put_inputs(self, in_map):
        import jax
        self._dev_inputs = [
            jax.device_put(np.asarray(in_map[n]), self.device)
            for n in self.in_names]
        jax.block_until_ready(self._dev_inputs)

    def launch(self):
        return self.jitted(*self._dev_inputs,
                           *[z.copy() for z in self.zero_outs])


def run_async(runners):
    import jax
    futs = [r.launch() for r in runners]
    for f in futs:
        jax.block_until_ready(f)
    return futs


def kernel(x, theta):
    import jax
    x = np.ascontiguousarray(x, dtype=np.float32)
    theta = np.ascontiguousarray(theta, dtype=np.float32)
    devs = jax.devices()[:8]

    if "l1" not in _cache:
        nc1 = _build_l1()
        _cache["l1"] = [Runner(nc1, devs[i]) for i in range(8)]
    l1r = _cache["l1"]
    for bidx in range(B):
        l1r[bidx].put_inputs(
            {"xs": np.ascontiguousarray(x[bidx].reshape(C, HW))})
    futs = run_async(l1r)
    xbb = np.ascontiguousarray(
        np.stack([np.asarray(futs[bidx][0]).reshape(HW) for bidx in range(B)]))

    key = theta.tobytes()
    if _cache.get("l2key") != key:
        core_batches, slotmaps = make_sched(theta)
        ncs = [_build_l2(cb) for cb in core_batches]
        _cache["l2"] = [Runner(ncs[i], devs[i]) for i in range(8)]
        _cache["l2meta"] = (core_batches, slotmaps)
        _cache["l2key"] = key
    l2r = _cache["l2"]
    core_batches, slotmaps = _cache["l2meta"]
    in_maps = _prep_in2(xbb, theta, core_batches)
    for i in range(8):
        l2r[i].put_inputs(in_maps[i])
    futs = run_async(l2r)
    out = np.zeros((B, B, H, W), np.float32)
    for ci in range(8):
        r = np.asarray(futs[ci][0]).astype(np.float32)
        r = r.reshape(-1, B, P)
        for si, sm in enumerate(slotmaps[ci]):
            if sm is None:
                continue
            o, row, xw = sm
            out[:, o, row, xw:xw + P] = r[si]
    return out
